# revision 1
# baseline (speedup 1.0000x reference)
"""MoE feed-forward (8 experts, top-2 routing) on 8 Trainium2 NeuronCores.

Strategy (expert parallelism):
  - Router runs on host with jax-CPU, replicating the reference's fp32 ops
    bit-for-bit (einsum + top_k + softmax) so expert selection matches.
  - Tokens are dispatched (gathered) per expert on host; each of the 8 cores
    runs one expert's SwiGLU FFN over its tokens:
        h = silu(x @ W1) * (x @ W2);  y = comb * (h @ W3)
    Stage 1 matmuls run as float32r (full PE rate, fp32 storage); h is stored
    bf16 in SBUF; stage 2 runs bf16 x bf16 with fp32 PSUM accumulation.
  - Host combines: out[token] += y_e rows (softmax weights already applied on
    device), plus the (comb @ b3) bias term.
"""

import sys
import types

for _p in ("/opt/trn_rl_repo", "/root/.axon_site/_ro/trn_rl_repo"):
    if _p not in sys.path:
        sys.path.append(_p)

import numpy as np
import ml_dtypes

import concourse.bass as bass
import concourse.mybir as mybir
import concourse.tile as tile
from concourse.bass_utils import run_bass_kernel_spmd

D_MODEL = 1024
D_FF = 4096
N_EXPERTS = 8
TOP_K = 2
P = 128
KO = D_MODEL // P  # 8 k-tiles over d_model
MF = D_FF // P  # 32 slices over d_ff

F32 = mybir.dt.float32
F32R = mybir.dt.float32r
BF16 = mybir.dt.bfloat16


# ---------------------------------------------------------------------------
# Workarounds for this container's toolchain
# ---------------------------------------------------------------------------
def _install_workarounds():
    # walrus here rejects >1 sync-wait on the TileContext-final Drain; split
    # the waits across a chain of single-wait drains.
    def _drain_and_barrier_split(self, tick_clock, wait_clock):
        drain_inst = self.nc.sync.drain()
        wait_clock.add_sem_waits(
            drain_inst.ins, tile.ScopedClock({None: tick_clock.global_clock})
        )
        si = drain_inst.ins.sync_info
        waits = list(si.on_wait) if si is not None else []
        if len(waits) > 1:
            si.on_wait = [waits[0]]
            for w in waits[1:]:
                d2 = self.nc.sync.drain()
                d2.ins.sync_info = mybir.SyncInfo(on_wait=[w], on_update=[])
        self.nc.all_engine_barrier()
        popped = self.nc._tile_sem_poison_stack.pop()
        assert popped is self._sem_poison
        self.nc.clear_and_free_semaphores(list(self.sems.allocated().values()))
        self.nc.all_engine_barrier()

    tile.TileContext._drain_and_barrier = _drain_and_barrier_split

    # antenv.axon_hooks is absent on this image; register the NTFF profile
    # hook from trn_agent_boot so trace=True works (no-op for trace=False).
    if "antenv.axon_hooks" not in sys.modules:
        try:
            from trn_agent_boot.trn_boot import _ntff_profile_via_ctypes

            hook = _ntff_profile_via_ctypes("/opt/axon/libaxon_pjrt.so")
        except Exception:
            hook = None
        mod = types.ModuleType("antenv.axon_hooks")
        mod.get_axon_ntff_profile_hook = lambda: hook
        mod.set_axon_ntff_profile_hook = lambda h: None
        sys.modules["antenv.axon_hooks"] = mod

    # artifact upload needs S3 creds we don't have; keep artifacts local.
    import concourse.bass_utils as bu

    bu.upload_artifacts = lambda tmpdir: "local://" + tmpdir

    # This walrus build accepts at most ONE sync-wait per non-DMA instruction
    # ("Too many sync wait commands"). Hoist extra waits onto single-wait
    # NoOps emitted just before the instruction on the same engine.
    import orjson

    def _split_multiwaits(bir: bytes) -> bytes:
        m = orjson.loads(bir)
        ctr = 0
        changed = False
        for f in m["functions"]:
            for blk in f["blocks"]:
                newinsts = []
                for inst in blk["instructions"]:
                    si = inst.get("sync_info")
                    if si and len(si.get("on_wait", [])) > 1:
                        waits = si["on_wait"]
                        for w in waits[:-1]:
                            ctr += 1
                            newinsts.append(
                                {
                                    "debug": inst.get("debug", 0),
                                    "engine": inst["engine"],
                                    "ins": [],
                                    "outs": [],
                                    "name": f"{inst['name']}_sw{ctr}",
                                    "opcode": "NoOp",
                                    "sync_info": {
                                        "on_wait": [w],
                                        "on_update": [],
                                    },
                                }
                            )
                        si["on_wait"] = [waits[-1]]
                        changed = True
                    newinsts.append(inst)
                blk["instructions"] = newinsts
        return orjson.dumps(m) if changed else bir

    _orig_tjb = bass.Bass.to_json_bytes

    def _to_json_bytes_split(self):
        return _split_multiwaits(_orig_tjb(self))

    bass.Bass.to_json_bytes = _to_json_bytes_split


_install_workarounds()


# ---------------------------------------------------------------------------
# Host-side router — replicates the reference router on jax-CPU
# ---------------------------------------------------------------------------
def _route(x, Wr, br):
    """Return comb [T, E] fp32 combine weights (0 for unselected experts) and
    top_idx [T, K] int — computed exactly as the reference does, on CPU."""
    import jax
    import jax.numpy as jnp

    cpu = jax.devices("cpu")[0]
    with jax.default_device(cpu):
        xj = jnp.asarray(np.asarray(x))
        logits = jnp.einsum("bsd,de->bse", xj, jnp.asarray(np.asarray(Wr)))
        logits = logits + jnp.asarray(np.asarray(br))
        top_vals, top_idx = jax.lax.top_k(logits, TOP_K)
        top_w = jax.nn.softmax(top_vals, axis=-1)
        comb = jnp.sum(
            jax.nn.one_hot(top_idx, N_EXPERTS, dtype=xj.dtype) * top_w[..., None],
            axis=-2,
        )
        comb_np = np.asarray(comb).reshape(-1, N_EXPERTS)
        idx_np = np.asarray(top_idx).reshape(-1, TOP_K)
    return comb_np, idx_np


def _token_blocks(tp):
    """Split tp (multiple of 64) into fp32r-friendly token blocks. Measured
    on HW: N=384 matmuls hit the ideal issue rate, N=512 run ~10% over, so
    prefer 384; everything must stay >=256 for full-rate float32r."""
    u = tp // 64
    if u <= 8:
        if u == 8:
            return [256, 256]
        return [tp]
    # Best measured schedule (tp=1088 -> [256, 448, 384], 408.1us):
    # the 256-lead block gates startup on a small xT DMA; fewer, larger
    # blocks beat more, ideal-rate ones ([384,384,320]=412.9us,
    # [256,256,256,320]=415.4us) because each extra block costs PSUM
    # group-transition overhead.
    blocks = [256]
    u -= 4
    while u:
        if 4 <= u <= 7:
            blocks.append(u * 64)
            u = 0
        elif u == 8:
            blocks += [256, 256]
            u = 0
        elif u == 9:
            blocks += [320, 256]
            u = 0
        else:
            nb = 7 if u - 7 >= 4 else 6
            blocks.append(nb * 64)
            u -= nb
    return blocks


# ---------------------------------------------------------------------------
# Device program (one expert per core, SPMD)
# ---------------------------------------------------------------------------
_prog_cache = {}
_FORCE_TP128 = False


def _build_program(tp, stage1_f32r=True):
    """Bass program for one expert FFN over tp (padded) tokens.

    Host-side array layouts (all pre-shuffled for contiguous DMA rows):
      xT   [P, KO, tp]      x gathered+transposed, fp32
      w1/w2 [MF, P, KO, P]  (m, p, ko, f) = W1[ko*128+p, m*128+f], fp32
      w3   [NQ, P, MF, QW]  (q, p, k, d) = W3[k*128+p, q*256+d], bf16
      comb [P, NTC]         (p, t) = weight of token t*128+p, fp32
      y    [tp, D_MODEL]    output, fp32
    """
    QW = 512
    NQ = D_MODEL // QW
    NTC = -(-tp // P)  # ceil: phase-2 token sub-blocks (last may be 64)

    nc = bass.Bass()
    s1dt = F32R if stage1_f32r else F32
    xT = nc.dram_tensor("xT", [P, KO, tp], s1dt, kind="ExternalInput")
    w1 = nc.dram_tensor("w1", [MF, P, KO, P], s1dt, kind="ExternalInput")
    w2 = nc.dram_tensor("w2", [MF, P, KO, P], s1dt, kind="ExternalInput")
    w3 = nc.dram_tensor("w3", [NQ, P, MF, QW], BF16, kind="ExternalInput")
    comb = nc.dram_tensor("comb", [P, NTC], F32, kind="ExternalInput")
    y = nc.dram_tensor("y", [tp, D_MODEL], F32, kind="ExternalOutput")

    blocks = _token_blocks(tp)
    bmax = max(blocks)
    tblocks = [(i * P, P) for i in range(tp // P)]
    if tp % P:
        tblocks.append((tp // P * P, tp % P))

    with tile.TileContext(nc) as tc:
        with (
            tc.tile_pool(name="persist", bufs=1) as persist,
            tc.tile_pool(name="w3p", bufs=2) as w3p,
            tc.tile_pool(name="wp", bufs=2) as wp,
            tc.tile_pool(name="sp", bufs=3) as sp,
            tc.tile_pool(name="yp", bufs=3) as yp,
            tc.tile_pool(name="psA", bufs=2, space="PSUM") as psA,
            tc.tile_pool(name="psB", bufs=2, space="PSUM") as psB,
            tc.tile_pool(name="psY", bufs=4, space="PSUM") as psY,
        ):
            # --- persistent SBUF tensors ---
            xT_sb = persist.tile([P, KO, tp], s1dt)
            h_sb = persist.tile([P, MF, tp], BF16)
            comb_sb = persist.tile([P, NTC], F32)
            nc.sync.dma_start(comb_sb[:], comb[:])
            # load xT in (block, ko) chunks so the first token block's
            # matmuls only wait on a few parallel ~1.5KB-row DMAs; blocks
            # past the first are emitted after m=0's weight loads so the
            # startup-critical DMAs all land in the first queue wave
            def _xt_block_dma(t0, nb, eng=None):
                for ko in range(KO):
                    (eng or nc.sync).dma_start(
                        xT_sb[:, ko, t0 : t0 + nb], xT[:, ko, t0 : t0 + nb]
                    )

            # first block via gpsimd/SWDGE: parallel trigger stream with the
            # sync-engine weight loads, halving startup trigger serialization
            _xt_block_dma(0, blocks[0], eng=nc.gpsimd)

            # --- phase 1: h = silu(x@W1) * (x@W2), stored bf16 ---
            prio_at_m = []
            for m in range(MF):
                prio_at_m.append(tc.cur_priority)
                w1t = wp.tile([P, KO, P], s1dt, tag="w1t")
                w2t = wp.tile([P, KO, P], s1dt, tag="w2t")
                # chunked loads (4 parallel DMAs each, 1KB contiguous rows):
                # single-queue DMA bandwidth would otherwise gate startup
                for kg in range(0, KO, 2):
                    nc.sync.dma_start(w1t[:, kg : kg + 2], w1[m, :, kg : kg + 2])
                    nc.sync.dma_start(w2t[:, kg : kg + 2], w2[m, :, kg : kg + 2])
                if m == 0:
                    t0 = blocks[0]
                    for nb in blocks[1:]:
                        _xt_block_dma(t0, nb)
                        t0 += nb
                t0 = 0
                for nb in blocks:
                    tsl = slice(t0, t0 + nb)
                    ps1_full = psA.tile([P, bmax], F32, tag="ps1", name="ps1")
                    ps2_full = psB.tile([P, bmax], F32, tag="ps2", name="ps2")
                    ps1 = ps1_full[:, :nb]
                    ps2 = ps2_full[:, :nb]
                    for ko in range(KO):
                        nc.tensor.matmul(
                            ps1,
                            w1t[:, ko],
                            xT_sb[:, ko, tsl],
                            start=(ko == 0),
                            stop=(ko == KO - 1),
                        )
                    for ko in range(KO):
                        nc.tensor.matmul(
                            ps2,
                            w2t[:, ko],
                            xT_sb[:, ko, tsl],
                            start=(ko == 0),
                            stop=(ko == KO - 1),
                        )
                    sil_full = sp.tile([P, bmax], F32, tag="sil", name="sil")
                    sil = sil_full[:, :nb]
                    nc.scalar.activation(
                        sil, ps1, mybir.ActivationFunctionType.Silu
                    )
                    nc.vector.tensor_mul(h_sb[:, m, tsl], sil, ps2)
                    t0 += nb

            # --- phase 2: y = comb * (h @ W3), d_model in two halves with
            # W3 double-buffered (bufs=2) and prefetched during phase 1 —
            # N=512 matmuls run at 0.416 ns/col vs 0.438 for N=256 ---
            for q in range(NQ):
                dsl = slice(q * QW, (q + 1) * QW)
                w3q = w3p.tile([P, MF, QW], BF16, tag="w3q")
                # schedule this half's W3 load as if issued mid-phase-1 so
                # it neither starves the startup DMAs nor arrives late
                prio_save = tc.cur_priority
                tc.cur_priority = prio_at_m[min(8 + 12 * q, MF - 1)]
                for kg in range(0, MF, 4):
                    nc.sync.dma_start(w3q[:, kg : kg + 4], w3[q, :, kg : kg + 4])
                tc.cur_priority = prio_save
                for t0, tb in tblocks:
                    psy_full = psY.tile([P, QW], F32, tag="psy", name="psy")
                    psy = psy_full[:tb]
                    tsl = slice(t0, t0 + tb)
                    for k in range(MF):
                        nc.tensor.matmul(
                            psy,
                            h_sb[:, k, tsl],
                            w3q[:, k],
                            start=(k == 0),
                            stop=(k == MF - 1),
                        )
                    ysb_full = yp.tile([P, QW], F32, tag="ysb", name="ysb")
                    ysb = ysb_full[:tb]
                    ti = t0 // P
                    nc.vector.tensor_scalar_mul(ysb, psy, comb_sb[:tb, ti : ti + 1])
                    # two half-width writes so the final store does not add
                    # a long single-queue tail
                    hq = QW // 2
                    nc.sync.dma_start(y[tsl, q * QW : q * QW + hq], ysb[:, :hq])
                    nc.sync.dma_start(y[tsl, q * QW + hq : (q + 1) * QW], ysb[:, hq:])
    return nc


def _get_program(tp, stage1_f32r=True):
    key = (tp, stage1_f32r)
    if key not in _prog_cache:
        _prog_cache[key] = _build_program(tp, stage1_f32r)
    return _prog_cache[key]


# ---------------------------------------------------------------------------
# Public entry point
# ---------------------------------------------------------------------------
def kernel(x, Wr, br, W1, b1, W2, b2, W3, b3):
    x = np.asarray(x)
    Wr = np.asarray(Wr)
    br = np.asarray(br)
    W1 = np.asarray(W1)
    b1 = np.asarray(b1)
    W2 = np.asarray(W2)
    b2 = np.asarray(b2)
    W3 = np.asarray(W3)
    b3 = np.asarray(b3)

    B, S, _ = x.shape
    T = B * S
    xf = np.ascontiguousarray(x.reshape(T, D_MODEL))

    if np.any(b1) or np.any(b2):
        raise NotImplementedError("nonzero b1/b2 not supported by this kernel")

    comb, top_idx = _route(x, Wr, br)

    # Dispatch: gather each expert's tokens (host all-to-all).
    sels = []
    for e in range(N_EXPERTS):
        sel = np.nonzero((top_idx == e).any(axis=1))[0]
        sels.append(sel)
    n_max = max(len(s) for s in sels)
    tp = max(512, -(-n_max // 64) * 64)  # pad to multiple of 64, >= 512
    if _FORCE_TP128:
        tp = max(512, -(-n_max // P) * P)
    ntc = -(-tp // P)

    # weight shuffles into DMA-friendly layouts (see _build_program docstring)
    w1d = W1.reshape(N_EXPERTS, KO, P, MF, P).transpose(0, 3, 2, 1, 4)
    w2d = W2.reshape(N_EXPERTS, KO, P, MF, P).transpose(0, 3, 2, 1, 4)
    w3d = (
        W3.astype(ml_dtypes.bfloat16)
        .reshape(N_EXPERTS, MF, P, 2, 512)
        .transpose(0, 3, 2, 1, 4)
    )

    in_maps = []
    for e in range(N_EXPERTS):
        sel = sels[e]
        n_e = len(sel)
        xT_e = np.zeros((P, KO, tp), dtype=np.float32)
        if n_e:
            xT_e[:, :, :n_e] = xf[sel].reshape(n_e, KO, P).transpose(2, 1, 0)
        comb_e = np.zeros(ntc * P, dtype=np.float32)
        if n_e:
            comb_e[:n_e] = comb[sel, e]
        in_maps.append(
            {
                "xT": xT_e,
                "w1": np.ascontiguousarray(w1d[e]),
                "w2": np.ascontiguousarray(w2d[e]),
                "w3": np.ascontiguousarray(w3d[e]),
                "comb": np.ascontiguousarray(comb_e.reshape(ntc, P).T),
            }
        )

    nc = _get_program(tp)
    try:
        res = run_bass_kernel_spmd(nc, in_maps, core_ids=list(range(N_EXPERTS)))
    except Exception:
        # transient NRT/axon device hiccups have been observed; retry once
        import time as _time

        _time.sleep(5)
        res = run_bass_kernel_spmd(nc, in_maps, core_ids=list(range(N_EXPERTS)))

    # Combine: scatter-add weighted expert outputs (weights already applied).
    out = np.zeros((T, D_MODEL), dtype=np.float32)
    for e in range(N_EXPERTS):
        sel = sels[e]
        if len(sel):
            out[sel] += res.results[e]["y"][: len(sel)]
    if np.any(b3):
        out += comb @ b3
    return out.reshape(B, S, D_MODEL)



# revision 3
# speedup vs baseline: 1.0526x; 1.0526x over previous
"""MoE feed-forward (8 experts, top-2 routing) on 8 Trainium2 NeuronCores.

Strategy (balanced expert parallelism, all-bf16):
  - Router runs on host with jax-CPU, replicating the reference's fp32 ops
    (einsum + top_k + softmax) so expert selection matches exactly.
  - Expert identity is pure data under SPMD: every core runs the same
    program over two fixed-size token segments (s1=544, s2=512 slots), and
    each core's in_map supplies whichever experts' weights its segments
    need.  The 4 most-loaded experts are split across two cores' segment-A
    slots, the 4 least-loaded across two cores' segment-B slots, so every
    core processes C = s1+s2 = 1056 token slots (vs 1088 + phase-2 padding
    for one-expert-per-core).
  - The top-2 softmax combine weight is folded into the W2-path activations
    on the host (y = (silu(x@W1) * ((comb*x)@W2)) @ W3 is linear in the
    W2-path input), so the device applies no per-token scaling at all and
    phase 2 is token-granular.
  - Phase 1: h = silu(x@W1) * (xv@W2), bf16 in / f32 PSUM / bf16 h.
  - Phase 2: y[dslice] = W3_slice.T @ h contraction over d_ff with h as the
    moving operand; y stored bf16, host does the scatter-add combine.
"""

import sys
import types

for _p in ("/opt/trn_rl_repo", "/root/.axon_site/_ro/trn_rl_repo"):
    if _p not in sys.path:
        sys.path.append(_p)

import numpy as np
import ml_dtypes

import concourse.bass as bass
import concourse.mybir as mybir
import concourse.tile as tile
from concourse.bass_utils import run_bass_kernel_spmd

D_MODEL = 1024
D_FF = 4096
N_EXPERTS = 8
TOP_K = 2
P = 128
KO = D_MODEL // P  # 8 k-tiles over d_model
MF = D_FF // P  # 32 slices over d_ff
NQ = D_MODEL // P  # 8 output d_model slices
QD = P

F32 = mybir.dt.float32
BF16 = mybir.dt.bfloat16


# ---------------------------------------------------------------------------
# Workarounds for this container's toolchain
# ---------------------------------------------------------------------------
def _install_workarounds():
    # walrus here rejects >1 sync-wait on the TileContext-final Drain; split
    # the waits across a chain of single-wait drains.
    def _drain_and_barrier_split(self, tick_clock, wait_clock):
        drain_inst = self.nc.sync.drain()
        wait_clock.add_sem_waits(
            drain_inst.ins, tile.ScopedClock({None: tick_clock.global_clock})
        )
        si = drain_inst.ins.sync_info
        waits = list(si.on_wait) if si is not None else []
        if len(waits) > 1:
            si.on_wait = [waits[0]]
            for w in waits[1:]:
                d2 = self.nc.sync.drain()
                d2.ins.sync_info = mybir.SyncInfo(on_wait=[w], on_update=[])
        self.nc.all_engine_barrier()
        popped = self.nc._tile_sem_poison_stack.pop()
        assert popped is self._sem_poison
        self.nc.clear_and_free_semaphores(list(self.sems.allocated().values()))
        self.nc.all_engine_barrier()

    tile.TileContext._drain_and_barrier = _drain_and_barrier_split

    # antenv.axon_hooks is absent on this image; register the NTFF profile
    # hook from trn_agent_boot so trace=True works (no-op for trace=False).
    if "antenv.axon_hooks" not in sys.modules:
        try:
            from trn_agent_boot.trn_boot import _ntff_profile_via_ctypes

            hook = _ntff_profile_via_ctypes("/opt/axon/libaxon_pjrt.so")
        except Exception:
            hook = None
        mod = types.ModuleType("antenv.axon_hooks")
        mod.get_axon_ntff_profile_hook = lambda: hook
        mod.set_axon_ntff_profile_hook = lambda h: None
        sys.modules["antenv.axon_hooks"] = mod

    # artifact upload needs S3 creds we don't have; keep artifacts local.
    import concourse.bass_utils as bu

    bu.upload_artifacts = lambda tmpdir: "local://" + tmpdir

    # This walrus build accepts at most ONE sync-wait per non-DMA instruction
    # ("Too many sync wait commands"). Hoist extra waits onto single-wait
    # NoOps emitted just before the instruction on the same engine.
    import orjson

    def _split_multiwaits(bir: bytes) -> bytes:
        m = orjson.loads(bir)
        ctr = 0
        changed = False
        for f in m["functions"]:
            for blk in f["blocks"]:
                newinsts = []
                for inst in blk["instructions"]:
                    si = inst.get("sync_info")
                    if si and len(si.get("on_wait", [])) > 1:
                        waits = si["on_wait"]
                        for w in waits[:-1]:
                            ctr += 1
                            newinsts.append(
                                {
                                    "debug": inst.get("debug", 0),
                                    "engine": inst["engine"],
                                    "ins": [],
                                    "outs": [],
                                    "name": f"{inst['name']}_sw{ctr}",
                                    "opcode": "NoOp",
                                    "sync_info": {
                                        "on_wait": [w],
                                        "on_update": [],
                                    },
                                }
                            )
                        si["on_wait"] = [waits[-1]]
                        changed = True
                    newinsts.append(inst)
                blk["instructions"] = newinsts
        return orjson.dumps(m) if changed else bir

    _orig_tjb = bass.Bass.to_json_bytes

    def _to_json_bytes_split(self):
        return _split_multiwaits(_orig_tjb(self))

    bass.Bass.to_json_bytes = _to_json_bytes_split


_install_workarounds()


# ---------------------------------------------------------------------------
# Host-side router — replicates the reference router on jax-CPU
# ---------------------------------------------------------------------------
def _route(x, Wr, br):
    """Return comb [T, E] fp32 combine weights (0 for unselected experts) and
    top_idx [T, K] int — computed exactly as the reference does, on CPU."""
    import jax
    import jax.numpy as jnp

    cpu = jax.devices("cpu")[0]
    with jax.default_device(cpu):
        xj = jnp.asarray(np.asarray(x))
        logits = jnp.einsum("bsd,de->bse", xj, jnp.asarray(np.asarray(Wr)))
        logits = logits + jnp.asarray(np.asarray(br))
        top_vals, top_idx = jax.lax.top_k(logits, TOP_K)
        top_w = jax.nn.softmax(top_vals, axis=-1)
        comb = jnp.sum(
            jax.nn.one_hot(top_idx, N_EXPERTS, dtype=xj.dtype) * top_w[..., None],
            axis=-2,
        )
        comb_np = np.asarray(comb).reshape(-1, N_EXPERTS)
        idx_np = np.asarray(top_idx).reshape(-1, TOP_K)
    return comb_np, idx_np


def _seg_blocks(s, first_m=False):
    """Column blocks for a segment of s tokens (PSUM caps N at 512). The
    first phase-1 m-iteration uses smaller leading blocks so the startup
    x DMAs gate less PE time."""
    out = []
    t0 = 0
    while t0 < s:
        nb = min(512, s - t0)
        if first_m and t0 == 0 and nb == 512:
            out += [(0, 256), (256, 256)]
            t0 = 512
            continue
        out.append((t0, nb))
        t0 += nb
    return out


# ---------------------------------------------------------------------------
# Device program (two expert segments per core, SPMD)
# ---------------------------------------------------------------------------
_prog_cache = {}


def _build_program(s1, s2):
    """Bass program over C = s1 + s2 token slots: segment A = [0, s1) runs
    expert "a" weights, segment B = [s1, s1+s2) expert "b" weights.

    Host-side array layouts (pre-shuffled for contiguous DMA rows):
      xu, xv [P, KO, C] bf16   (p, ko, t) = x[t, ko*128+p]; xv comb-scaled
      w1a/w1b/w2a/w2b [MF, P, KO, P] bf16  (m, p, ko, f) = W[ko*128+p, m*128+f]
      w3a/w3b [NQ, P, MF, QD] bf16         (q, p, k, d) = W3[k*128+p, q*128+d]
      y [NQ, P, C] bf16        (q, d, t) output, transposed layout
    """
    C = s1 + s2
    nc = bass.Bass()
    xu = nc.dram_tensor("xu", [P, KO, C], BF16, kind="ExternalInput")
    xv = nc.dram_tensor("xv", [P, KO, C], BF16, kind="ExternalInput")
    w1a = nc.dram_tensor("w1a", [MF, P, KO, P], BF16, kind="ExternalInput")
    w2a = nc.dram_tensor("w2a", [MF, P, KO, P], BF16, kind="ExternalInput")
    w1b = nc.dram_tensor("w1b", [MF, P, KO, P], BF16, kind="ExternalInput")
    w2b = nc.dram_tensor("w2b", [MF, P, KO, P], BF16, kind="ExternalInput")
    w3a = nc.dram_tensor("w3a", [NQ, P, MF, QD], BF16, kind="ExternalInput")
    w3b = nc.dram_tensor("w3b", [NQ, P, MF, QD], BF16, kind="ExternalInput")
    y = nc.dram_tensor("y", [NQ, P, C], BF16, kind="ExternalOutput")

    segs = [(0, s1), (s1, s2)]

    with tile.TileContext(nc) as tc:
        with (
            tc.tile_pool(name="persist", bufs=1) as persist,
            tc.tile_pool(name="wp", bufs=4) as wp,
            tc.tile_pool(name="w3p", bufs=2) as w3p,
            tc.tile_pool(name="sp", bufs=3) as sp,
            tc.tile_pool(name="yp", bufs=3) as yp,
            tc.tile_pool(name="psA", bufs=2, space="PSUM") as psA,
            tc.tile_pool(name="psB", bufs=2, space="PSUM") as psB,
            tc.tile_pool(name="psY", bufs=3, space="PSUM") as psY,
        ):
            xu_sb = persist.tile([P, KO, C], BF16)
            xv_sb = persist.tile([P, KO, C], BF16)
            h_sb = persist.tile([P, MF, C], BF16)

            def _x_dma(t0, nb, eng):
                for t in (xu_sb, xu), (xv_sb, xv):
                    sb, dr = t
                    for kg in range(0, KO, 4):
                        eng.dma_start(
                            sb[:, kg : kg + 4, t0 : t0 + nb],
                            dr[:, kg : kg + 4, t0 : t0 + nb],
                        )

            # startup-critical x: first block of segment A via gpsimd/SWDGE
            # (parallel trigger stream with the sync-engine weight loads)
            _x_dma(0, 256, nc.gpsimd)

            w1_dr = {0: w1a, 1: w1b}
            w2_dr = {0: w2a, 1: w2b}

            prio_at_m = []
            for m in range(MF):
                prio_at_m.append(tc.cur_priority)
                for si, (off, s) in enumerate(segs):
                    w1t = wp.tile([P, KO, P], BF16, tag=f"w1t{si}")
                    w2t = wp.tile([P, KO, P], BF16, tag=f"w2t{si}")
                    for kg in range(0, KO, 4):
                        nc.sync.dma_start(
                            w1t[:, kg : kg + 4], w1_dr[si][m, :, kg : kg + 4]
                        )
                        nc.sync.dma_start(
                            w2t[:, kg : kg + 4], w2_dr[si][m, :, kg : kg + 4]
                        )
                    if m == 0 and si == 0:
                        # rest of x, queued behind the m=0 weight loads
                        _x_dma(256, 256, nc.gpsimd)
                        _x_dma(512, C - 512, nc.gpsimd)
                    for t0, nb in _seg_blocks(s, first_m=(m == 0)):
                        tsl = slice(off + t0, off + t0 + nb)
                        ps1_f = psA.tile([P, nb], F32, tag="ps1", name="ps1",
                                         padded_shape=[P, 512])
                        ps2_f = psB.tile([P, nb], F32, tag="ps2", name="ps2",
                                         padded_shape=[P, 512])
                        for ko in range(KO):
                            nc.tensor.matmul(
                                ps1_f,
                                w1t[:, ko],
                                xu_sb[:, ko, tsl],
                                start=(ko == 0),
                                stop=(ko == KO - 1),
                            )
                        for ko in range(KO):
                            nc.tensor.matmul(
                                ps2_f,
                                w2t[:, ko],
                                xv_sb[:, ko, tsl],
                                start=(ko == 0),
                                stop=(ko == KO - 1),
                            )
                        sil = sp.tile([P, nb], F32, tag="sil", name="sil",
                                      padded_shape=[P, 512])
                        nc.scalar.activation(
                            sil, ps1_f, mybir.ActivationFunctionType.Silu
                        )
                        nc.vector.tensor_mul(h_sb[:, m, tsl], sil, ps2_f)

            # --- phase 2: y[q] = h.T @ W3[:, q*128:(q+1)*128], h moving ---
            w3_dr = {0: w3a, 1: w3b}
            for q in range(NQ):
                w3t = {}
                # prefetch this q's W3 as if issued mid-phase-1
                prio_save = tc.cur_priority
                if q < 2:
                    tc.cur_priority = prio_at_m[min(18 + 7 * q, MF - 1)]
                for si in range(2):
                    w3t[si] = w3p.tile(
                        [P, MF, QD], BF16, tag=f"w3t{si}", name=f"w3t{si}"
                    )
                    for kg in range(0, MF, 8):
                        nc.sync.dma_start(
                            w3t[si][:, kg : kg + 8], w3_dr[si][q, :, kg : kg + 8]
                        )
                tc.cur_priority = prio_save
                for si, (off, s) in enumerate(segs):
                    for t0, nb in _seg_blocks(s):
                        tsl = slice(off + t0, off + t0 + nb)
                        psy_f = psY.tile([P, nb], F32, tag="psy", name="psy",
                                         padded_shape=[P, 512])
                        for k in range(MF):
                            nc.tensor.matmul(
                                psy_f,
                                w3t[si][:, k],
                                h_sb[:, k, tsl],
                                start=(k == 0),
                                stop=(k == MF - 1),
                            )
                        ysb = yp.tile([P, nb], BF16, tag="ysb", name="ysb",
                                      padded_shape=[P, 512])
                        nc.vector.tensor_copy(ysb, psy_f)
                        nc.gpsimd.dma_start(y[q, :, tsl], ysb)
    return nc


def _get_program(s1, s2):
    key = (s1, s2)
    if key not in _prog_cache:
        _prog_cache[key] = _build_program(s1, s2)
    return _prog_cache[key]


def _ceil64(n):
    return max(64, -(-n // 64) * 64)


# ---------------------------------------------------------------------------
# Public entry point
# ---------------------------------------------------------------------------
def kernel(x, Wr, br, W1, b1, W2, b2, W3, b3):
    x = np.asarray(x)
    Wr = np.asarray(Wr)
    br = np.asarray(br)
    W1 = np.asarray(W1)
    b1 = np.asarray(b1)
    W2 = np.asarray(W2)
    b2 = np.asarray(b2)
    W3 = np.asarray(W3)
    b3 = np.asarray(b3)

    B, S, _ = x.shape
    T = B * S
    xf = np.ascontiguousarray(x.reshape(T, D_MODEL))

    if np.any(b1) or np.any(b2):
        raise NotImplementedError("nonzero b1/b2 not supported by this kernel")

    comb, top_idx = _route(x, Wr, br)

    # Dispatch: gather each expert's tokens (host all-to-all).
    sels = []
    for e in range(N_EXPERTS):
        sel = np.nonzero((top_idx == e).any(axis=1))[0]
        sels.append(sel)
    counts = np.array([len(s) for s in sels])
    order = np.argsort(-counts, kind="stable")
    big, small = order[:4], order[4:]
    s1 = _ceil64(int(counts[big].max()) // 2 + (int(counts[big].max()) % 2 > 0))
    s2 = _ceil64(int(counts[small].max()) // 2 + (int(counts[small].max()) % 2 > 0))
    C = s1 + s2

    # weight shuffles into DMA-friendly bf16 layouts (see _build_program)
    bf16 = ml_dtypes.bfloat16
    w1d = W1.reshape(N_EXPERTS, KO, P, MF, P).transpose(0, 3, 2, 1, 4).astype(bf16)
    w2d = W2.reshape(N_EXPERTS, KO, P, MF, P).transpose(0, 3, 2, 1, 4).astype(bf16)
    w3d = W3.reshape(N_EXPERTS, MF, P, NQ, QD).transpose(0, 3, 2, 1, 4).astype(bf16)

    # core 2i / 2i+1 share big[i] in segment A and small[i] in segment B
    seg_tok = {}  # core -> [(expert, tokens, off, size)]
    in_maps = []
    for c in range(8):
        i, half = divmod(c, 2)
        eb, es = int(big[i]), int(small[i])
        tokA = sels[eb][half * s1 : (half + 1) * s1]
        tokB = sels[es][half * s2 : (half + 1) * s2]
        seg_tok[c] = [(eb, tokA, 0), (es, tokB, s1)]

        xu_c = np.zeros((P, KO, C), dtype=bf16)
        xv_c = np.zeros((P, KO, C), dtype=bf16)
        for e, toks, off in seg_tok[c]:
            n = len(toks)
            if not n:
                continue
            xs = xf[toks]
            xu_c[:, :, off : off + n] = (
                xs.astype(bf16).reshape(n, KO, P).transpose(2, 1, 0)
            )
            xv_c[:, :, off : off + n] = (
                (xs * comb[toks, e][:, None])
                .astype(bf16)
                .reshape(n, KO, P)
                .transpose(2, 1, 0)
            )
        in_maps.append(
            {
                "xu": xu_c,
                "xv": xv_c,
                "w1a": w1d[eb],
                "w2a": w2d[eb],
                "w1b": w1d[es],
                "w2b": w2d[es],
                "w3a": w3d[eb],
                "w3b": w3d[es],
            }
        )

    nc = _get_program(s1, s2)
    try:
        res = run_bass_kernel_spmd(nc, in_maps, core_ids=list(range(N_EXPERTS)))
    except Exception:
        # transient NRT/axon device hiccups have been observed; retry once
        import time as _time

        _time.sleep(5)
        res = run_bass_kernel_spmd(nc, in_maps, core_ids=list(range(N_EXPERTS)))

    # Combine: scatter-add expert outputs (softmax weights already folded in).
    out = np.zeros((T, D_MODEL), dtype=np.float32)
    for c in range(8):
        yc = np.asarray(res.results[c]["y"], dtype=np.float32)  # [NQ, P, C]
        yt = yc.transpose(2, 0, 1).reshape(C, D_MODEL)
        for e, toks, off in seg_tok[c]:
            n = len(toks)
            if n:
                out[toks] += yt[off : off + n]
    if np.any(b3):
        out += comb @ b3
    return out.reshape(B, S, D_MODEL)


# revision 7
# speedup vs baseline: 1.0580x; 1.0052x over previous
"""MoE feed-forward (8 experts, top-2 routing) on 8 Trainium2 NeuronCores.

Strategy (balanced expert parallelism, all-bf16):
  - Router runs on host with jax-CPU, replicating the reference's fp32 ops
    (einsum + top_k + softmax) so expert selection matches exactly.
  - Expert identity is pure data under SPMD: every core runs the same
    program over two fixed-size token segments (s1=544, s2=512 slots), and
    each core's in_map supplies whichever experts' weights its segments
    need.  The 4 most-loaded experts are split across two cores' segment-A
    slots, the 4 least-loaded across two cores' segment-B slots, so every
    core processes C = s1+s2 = 1056 token slots (vs 1088 + phase-2 padding
    for one-expert-per-core).
  - The top-2 softmax combine weight is folded into the W2-path activations
    on the host (y = (silu(x@W1) * ((comb*x)@W2)) @ W3 is linear in the
    W2-path input), so the device applies no per-token scaling at all and
    phase 2 is token-granular.
  - Phase 1: h = silu(x@W1) * (xv@W2), bf16 in / f32 PSUM / bf16 h.
  - Phase 2: y[dslice] = W3_slice.T @ h contraction over d_ff with h as the
    moving operand; y stored bf16, host does the scatter-add combine.
"""

import sys
import types

for _p in ("/opt/trn_rl_repo", "/root/.axon_site/_ro/trn_rl_repo"):
    if _p not in sys.path:
        sys.path.append(_p)

import numpy as np
import ml_dtypes

import concourse.bass as bass
import concourse.mybir as mybir
import concourse.tile as tile
from concourse.bass_utils import run_bass_kernel_spmd

D_MODEL = 1024
D_FF = 4096
N_EXPERTS = 8
TOP_K = 2
P = 128
KO = D_MODEL // P  # 8 k-tiles over d_model
MF = D_FF // P  # 32 slices over d_ff
NQ = D_MODEL // P  # 8 output d_model slices
QD = P

F32 = mybir.dt.float32
BF16 = mybir.dt.bfloat16


# ---------------------------------------------------------------------------
# Workarounds for this container's toolchain
# ---------------------------------------------------------------------------
def _install_workarounds():
    # walrus here rejects >1 sync-wait on the TileContext-final Drain; split
    # the waits across a chain of single-wait drains.
    def _drain_and_barrier_split(self, tick_clock, wait_clock):
        drain_inst = self.nc.sync.drain()
        wait_clock.add_sem_waits(
            drain_inst.ins, tile.ScopedClock({None: tick_clock.global_clock})
        )
        si = drain_inst.ins.sync_info
        waits = list(si.on_wait) if si is not None else []
        if len(waits) > 1:
            si.on_wait = [waits[0]]
            for w in waits[1:]:
                d2 = self.nc.sync.drain()
                d2.ins.sync_info = mybir.SyncInfo(on_wait=[w], on_update=[])
        self.nc.all_engine_barrier()
        popped = self.nc._tile_sem_poison_stack.pop()
        assert popped is self._sem_poison
        self.nc.clear_and_free_semaphores(list(self.sems.allocated().values()))
        self.nc.all_engine_barrier()

    tile.TileContext._drain_and_barrier = _drain_and_barrier_split

    # antenv.axon_hooks is absent on this image; register the NTFF profile
    # hook from trn_agent_boot so trace=True works (no-op for trace=False).
    if "antenv.axon_hooks" not in sys.modules:
        try:
            from trn_agent_boot.trn_boot import _ntff_profile_via_ctypes

            hook = _ntff_profile_via_ctypes("/opt/axon/libaxon_pjrt.so")
        except Exception:
            hook = None
        mod = types.ModuleType("antenv.axon_hooks")
        mod.get_axon_ntff_profile_hook = lambda: hook
        mod.set_axon_ntff_profile_hook = lambda h: None
        sys.modules["antenv.axon_hooks"] = mod

    # artifact upload needs S3 creds we don't have; keep artifacts local.
    import concourse.bass_utils as bu

    bu.upload_artifacts = lambda tmpdir: "local://" + tmpdir

    # This walrus build accepts at most ONE sync-wait per non-DMA instruction
    # ("Too many sync wait commands"). Hoist extra waits onto single-wait
    # NoOps emitted just before the instruction on the same engine.
    import orjson

    def _split_multiwaits(bir: bytes) -> bytes:
        m = orjson.loads(bir)
        ctr = 0
        changed = False
        for f in m["functions"]:
            for blk in f["blocks"]:
                newinsts = []
                for inst in blk["instructions"]:
                    si = inst.get("sync_info")
                    if si and len(si.get("on_wait", [])) > 1:
                        waits = si["on_wait"]
                        for w in waits[:-1]:
                            ctr += 1
                            newinsts.append(
                                {
                                    "debug": inst.get("debug", 0),
                                    "engine": inst["engine"],
                                    "ins": [],
                                    "outs": [],
                                    "name": f"{inst['name']}_sw{ctr}",
                                    "opcode": "NoOp",
                                    "sync_info": {
                                        "on_wait": [w],
                                        "on_update": [],
                                    },
                                }
                            )
                        si["on_wait"] = [waits[-1]]
                        changed = True
                    newinsts.append(inst)
                blk["instructions"] = newinsts
        return orjson.dumps(m) if changed else bir

    _orig_tjb = bass.Bass.to_json_bytes

    def _to_json_bytes_split(self):
        return _split_multiwaits(_orig_tjb(self))

    bass.Bass.to_json_bytes = _to_json_bytes_split


_install_workarounds()


# ---------------------------------------------------------------------------
# Host-side router — replicates the reference router on jax-CPU
# ---------------------------------------------------------------------------
def _route(x, Wr, br):
    """Return comb [T, E] fp32 combine weights (0 for unselected experts) and
    top_idx [T, K] int — computed exactly as the reference does, on CPU."""
    import jax
    import jax.numpy as jnp

    cpu = jax.devices("cpu")[0]
    with jax.default_device(cpu):
        xj = jnp.asarray(np.asarray(x))
        logits = jnp.einsum("bsd,de->bse", xj, jnp.asarray(np.asarray(Wr)))
        logits = logits + jnp.asarray(np.asarray(br))
        top_vals, top_idx = jax.lax.top_k(logits, TOP_K)
        top_w = jax.nn.softmax(top_vals, axis=-1)
        comb = jnp.sum(
            jax.nn.one_hot(top_idx, N_EXPERTS, dtype=xj.dtype) * top_w[..., None],
            axis=-2,
        )
        comb_np = np.asarray(comb).reshape(-1, N_EXPERTS)
        idx_np = np.asarray(top_idx).reshape(-1, TOP_K)
    return comb_np, idx_np


def _seg_blocks(s, first_m=False):
    """Column blocks for a segment of s tokens (PSUM caps N at 512). The
    first phase-1 m-iteration uses smaller leading blocks so the startup
    x DMAs gate less PE time."""
    out = []
    t0 = 0
    while t0 < s:
        nb = min(512, s - t0)
        if first_m and t0 == 0 and nb == 512:
            out += [(0, 256), (256, 256)]
            t0 = 512
            continue
        out.append((t0, nb))
        t0 += nb
    return out


# ---------------------------------------------------------------------------
# Device program (two expert segments per core, SPMD)
# ---------------------------------------------------------------------------
_prog_cache = {}


def _build_program(s1, s2):
    """Bass program over C = s1 + s2 token slots: segment A = [0, s1) runs
    expert "a" weights, segment B = [s1, s1+s2) expert "b" weights.

    Host-side array layouts (pre-shuffled for contiguous DMA rows):
      xu, xv [P, KO, C] bf16   (p, ko, t) = x[t, ko*128+p]; xv comb-scaled
      w1a/w1b/w2a/w2b [MF, P, KO, P] bf16  (m, p, ko, f) = W[ko*128+p, m*128+f]
      w3a/w3b [NQ, P, MF, QD] bf16         (q, p, k, d) = W3[k*128+p, q*128+d]
      y [NQ, P, C] bf16        (q, d, t) output, transposed layout
    """
    C = s1 + s2
    nc = bass.Bass()
    xu = nc.dram_tensor("xu", [P, KO, C], BF16, kind="ExternalInput")
    xv = nc.dram_tensor("xv", [P, KO, C], BF16, kind="ExternalInput")
    w1a = nc.dram_tensor("w1a", [MF, P, KO, P], BF16, kind="ExternalInput")
    w2a = nc.dram_tensor("w2a", [MF, P, KO, P], BF16, kind="ExternalInput")
    w1b = nc.dram_tensor("w1b", [MF, P, KO, P], BF16, kind="ExternalInput")
    w2b = nc.dram_tensor("w2b", [MF, P, KO, P], BF16, kind="ExternalInput")
    w3a = nc.dram_tensor("w3a", [NQ, P, MF, QD], BF16, kind="ExternalInput")
    w3b = nc.dram_tensor("w3b", [NQ, P, MF, QD], BF16, kind="ExternalInput")
    y = nc.dram_tensor("y", [NQ, P, C], BF16, kind="ExternalOutput")

    segs = [(0, s1), (s1, s2)]

    with tile.TileContext(nc) as tc:
        with (
            tc.tile_pool(name="persist", bufs=1) as persist,
            tc.tile_pool(name="wp", bufs=4) as wp,
            tc.tile_pool(name="w3p", bufs=2) as w3p,
            tc.tile_pool(name="sp", bufs=3) as sp,
            tc.tile_pool(name="yp", bufs=3) as yp,
            tc.tile_pool(name="psA", bufs=2, space="PSUM") as psA,
            tc.tile_pool(name="psB", bufs=2, space="PSUM") as psB,
            tc.tile_pool(name="psY", bufs=3, space="PSUM") as psY,
        ):
            xu_sb = persist.tile([P, KO, C], BF16)
            xv_sb = persist.tile([P, KO, C], BF16)
            h_sb = persist.tile([P, MF, C], BF16)

            def _x_dma(t0, nb, engs):
                i = 0
                for sb, dr in (xu_sb, xu), (xv_sb, xv):
                    for kg in range(0, KO, 4):
                        engs[i % len(engs)].dma_start(
                            sb[:, kg : kg + 4, t0 : t0 + nb],
                            dr[:, kg : kg + 4, t0 : t0 + nb],
                        )
                        i += 1

            # startup-critical x: first block of segment A spread over the
            # gpsimd/vector/scalar trigger queues, in parallel with the
            # sync-engine weight loads (gpsimd DMA triggers cost ~650ns each,
            # so a single engine would serialize the startup transfers)
            _x_dma(0, 256, [nc.gpsimd, nc.scalar, nc.gpsimd, nc.scalar])

            w1_dr = {0: w1a, 1: w1b}
            w2_dr = {0: w2a, 1: w2b}

            # (m, segment) processing order: lead with two segment-A
            # m-iterations so the segment-B x DMAs get ~7µs to land
            mseg = [(0, 0), (1, 0), (0, 1), (1, 1)]
            mseg += [(m, si) for m in range(2, MF) for si in range(2)]

            prio_at_m = [None] * MF
            for m, si in mseg:
                if prio_at_m[m] is None:
                    prio_at_m[m] = tc.cur_priority
                off, s = segs[si]
                w1t = wp.tile([P, KO, P], BF16, tag=f"w1t{si}", name="w1t")
                w2t = wp.tile([P, KO, P], BF16, tag=f"w2t{si}", name="w2t")
                for kg in range(0, KO, 4):
                    nc.sync.dma_start(
                        w1t[:, kg : kg + 4], w1_dr[si][m, :, kg : kg + 4]
                    )
                    nc.sync.dma_start(
                        w2t[:, kg : kg + 4], w2_dr[si][m, :, kg : kg + 4]
                    )
                if (m, si) == (0, 0):
                    # rest of x, triggered behind the startup block
                    _x_dma(256, 256, [nc.scalar, nc.gpsimd, nc.scalar, nc.gpsimd])
                    _x_dma(
                        512, C - 512, [nc.gpsimd, nc.scalar, nc.gpsimd, nc.scalar]
                    )
                for t0, nb in _seg_blocks(s, first_m=(m == 0)):
                    tsl = slice(off + t0, off + t0 + nb)
                    ps1_f = psA.tile([P, nb], F32, tag="ps1", name="ps1",
                                     padded_shape=[P, 512])
                    ps2_f = psB.tile([P, nb], F32, tag="ps2", name="ps2",
                                     padded_shape=[P, 512])
                    for ko in range(KO):
                        nc.tensor.matmul(
                            ps1_f,
                            w1t[:, ko],
                            xu_sb[:, ko, tsl],
                            start=(ko == 0),
                            stop=(ko == KO - 1),
                        )
                    for ko in range(KO):
                        nc.tensor.matmul(
                            ps2_f,
                            w2t[:, ko],
                            xv_sb[:, ko, tsl],
                            start=(ko == 0),
                            stop=(ko == KO - 1),
                        )
                    sil = sp.tile([P, nb], F32, tag="sil", name="sil",
                                  padded_shape=[P, 512])
                    nc.scalar.activation(
                        sil, ps1_f, mybir.ActivationFunctionType.Silu
                    )
                    nc.vector.tensor_mul(h_sb[:, m, tsl], sil, ps2_f)

            # --- phase 2: y[q] = h.T @ W3[:, q*128:(q+1)*128], h moving ---
            w3_dr = {0: w3a, 1: w3b}
            for q in range(NQ):
                w3t = {}
                # prefetch this q's W3 as if issued mid-phase-1
                prio_save = tc.cur_priority
                if q < 2:
                    tc.cur_priority = prio_at_m[min(18 + 7 * q, MF - 1)]
                for si in range(2):
                    w3t[si] = w3p.tile(
                        [P, MF, QD], BF16, tag=f"w3t{si}", name=f"w3t{si}"
                    )
                    for kg in range(0, MF, 8):
                        nc.sync.dma_start(
                            w3t[si][:, kg : kg + 8], w3_dr[si][q, :, kg : kg + 8]
                        )
                tc.cur_priority = prio_save
                # biggest chains first, the 32-token remainder chain last so
                # the final store + drain tail is as small as possible
                chains = []
                for si, (off, s) in enumerate(segs):
                    for t0, nb in _seg_blocks(s):
                        chains.append((si, off + t0, nb))
                chains.sort(key=lambda c: -c[2])
                for si, t0, nb in chains:
                    tsl = slice(t0, t0 + nb)
                    psy_f = psY.tile([P, nb], F32, tag="psy", name="psy",
                                     padded_shape=[P, 512])
                    for k in range(MF):
                        nc.tensor.matmul(
                            psy_f,
                            w3t[si][:, k],
                            h_sb[:, k, tsl],
                            start=(k == 0),
                            stop=(k == MF - 1),
                        )
                    ysb = yp.tile([P, nb], BF16, tag="ysb", name="ysb",
                                  padded_shape=[P, 512])
                    nc.vector.tensor_copy(ysb, psy_f)
                    nc.sync.dma_start(y[q, :, tsl], ysb)
    return nc


def _get_program(s1, s2):
    key = (s1, s2)
    if key not in _prog_cache:
        _prog_cache[key] = _build_program(s1, s2)
    return _prog_cache[key]


def _ceil64(n):
    return max(64, -(-n // 64) * 64)


# ---------------------------------------------------------------------------
# Public entry point
# ---------------------------------------------------------------------------
def kernel(x, Wr, br, W1, b1, W2, b2, W3, b3):
    x = np.asarray(x)
    Wr = np.asarray(Wr)
    br = np.asarray(br)
    W1 = np.asarray(W1)
    b1 = np.asarray(b1)
    W2 = np.asarray(W2)
    b2 = np.asarray(b2)
    W3 = np.asarray(W3)
    b3 = np.asarray(b3)

    B, S, _ = x.shape
    T = B * S
    xf = np.ascontiguousarray(x.reshape(T, D_MODEL))

    if np.any(b1) or np.any(b2):
        raise NotImplementedError("nonzero b1/b2 not supported by this kernel")

    comb, top_idx = _route(x, Wr, br)

    # Dispatch: gather each expert's tokens (host all-to-all).
    sels = []
    for e in range(N_EXPERTS):
        sel = np.nonzero((top_idx == e).any(axis=1))[0]
        sels.append(sel)
    counts = np.array([len(s) for s in sels])
    order = np.argsort(-counts, kind="stable")
    big, small = order[:4], order[4:]
    s1 = _ceil64(int(counts[big].max()) // 2 + (int(counts[big].max()) % 2 > 0))
    s2 = _ceil64(int(counts[small].max()) // 2 + (int(counts[small].max()) % 2 > 0))
    C = s1 + s2

    # weight shuffles into DMA-friendly bf16 layouts (see _build_program)
    bf16 = ml_dtypes.bfloat16
    w1d = W1.reshape(N_EXPERTS, KO, P, MF, P).transpose(0, 3, 2, 1, 4).astype(bf16)
    w2d = W2.reshape(N_EXPERTS, KO, P, MF, P).transpose(0, 3, 2, 1, 4).astype(bf16)
    w3d = W3.reshape(N_EXPERTS, MF, P, NQ, QD).transpose(0, 3, 2, 1, 4).astype(bf16)

    # core 2i / 2i+1 share big[i] in segment A and small[i] in segment B
    seg_tok = {}  # core -> [(expert, tokens, off, size)]
    in_maps = []
    for c in range(8):
        i, half = divmod(c, 2)
        eb, es = int(big[i]), int(small[i])
        tokA = sels[eb][half * s1 : (half + 1) * s1]
        tokB = sels[es][half * s2 : (half + 1) * s2]
        seg_tok[c] = [(eb, tokA, 0), (es, tokB, s1)]

        xu_c = np.zeros((P, KO, C), dtype=bf16)
        xv_c = np.zeros((P, KO, C), dtype=bf16)
        for e, toks, off in seg_tok[c]:
            n = len(toks)
            if not n:
                continue
            xs = xf[toks]
            xu_c[:, :, off : off + n] = (
                xs.astype(bf16).reshape(n, KO, P).transpose(2, 1, 0)
            )
            xv_c[:, :, off : off + n] = (
                (xs * comb[toks, e][:, None])
                .astype(bf16)
                .reshape(n, KO, P)
                .transpose(2, 1, 0)
            )
        in_maps.append(
            {
                "xu": xu_c,
                "xv": xv_c,
                "w1a": w1d[eb],
                "w2a": w2d[eb],
                "w1b": w1d[es],
                "w2b": w2d[es],
                "w3a": w3d[eb],
                "w3b": w3d[es],
            }
        )

    nc = _get_program(s1, s2)
    try:
        res = run_bass_kernel_spmd(nc, in_maps, core_ids=list(range(N_EXPERTS)))
    except Exception:
        # transient NRT/axon device hiccups have been observed; retry once
        import time as _time

        _time.sleep(5)
        res = run_bass_kernel_spmd(nc, in_maps, core_ids=list(range(N_EXPERTS)))

    # Combine: scatter-add expert outputs (softmax weights already folded in).
    out = np.zeros((T, D_MODEL), dtype=np.float32)
    for c in range(8):
        yc = np.asarray(res.results[c]["y"], dtype=np.float32)  # [NQ, P, C]
        yt = yc.transpose(2, 0, 1).reshape(C, D_MODEL)
        for e, toks, off in seg_tok[c]:
            n = len(toks)
            if n:
                out[toks] += yt[off : off + n]
    if np.any(b3):
        out += comb @ b3
    return out.reshape(B, S, D_MODEL)
